# revision 35
# baseline (speedup 1.0000x reference)
"""Conditional contrastive loss on 8 TRN2 NeuronCores (Bass/Tile).

Strategy (data-parallel over rows, per sharding hint):
  - Each core owns 512 rows (of 4096) of inst_embed ("x") and proxy ("p").
  - The host row-normalizes x and p in fp32 and ships fp8(e4m3) operands
    in the exact on-chip layout: the full normalized xn^T (matmul rhs,
    k-chunked, one tile per 2048-column group), the core's own xn/pn
    columns (matmul lhsT, both matrices in one 4KB-row tensor), and the
    core's pre-gathered positive-selection mask rows negative_mask[labels]
    (fp8; 0/1 exact). This removes the entire on-device normalization
    pipeline, and every DMA moves fully-contiguous 4KB rows (the DMA
    fabric moves fixed ~158ns 4KB packet-slots across 16 engines).
  - Similarity rows sim[i, j] for the core's i-block: fp8 DoubleRow
    matmuls (2 contraction rows per PE cell -> K=256 per instruction)
    accumulated in PSUM, 2048 columns per PSUM group, double-buffered.
    A zero-matmul warmup stream keeps the PE HAM-warm through the DMA
    preamble (the HAM reaches 8/8 mid-warmup and must not re-throttle
    before the first data lands).
  - exp((sim-margin)/T) on the scalar engine straight out of PSUM with
    accum_out = per-group row sums -> denominator; z to SBUF in bf16.
  - numerator = scalar_tensor_tensor(z * mask) on DVE with accum_out.
    The DVE 1x fused op is the steady-state gate (~2.21us per 2048-col
    group); TT/TS alternatives measure slower because the fp8 and accum
    variants have no 2x uops.
  - Device emits raw per-group (den, num) row sums (p-major [128, 32]
    f32, one DMA); the host does the final group-sum/log/mean.
"""
import numpy as np
import ml_dtypes

import concourse.bacc as bacc
import concourse.tile as tile
from concourse import mybir, bass_utils

N_FULL = 4096
D = 512
N_CORES = 8
RP = N_FULL // N_CORES  # rows per core = 512
P = 128                 # SBUF partitions
KC = D // P             # 128-row contraction chunks = 4
JT = 512                # columns per PSUM bank
JG = 2048               # columns per PSUM group (4 banks)
NG = N_FULL // JG       # groups per (i-tile, matrix) = 2
IT = RP // P            # i-tiles per core = 4

F32 = mybir.dt.float32
BF16 = mybir.dt.bfloat16
F8 = mybir.dt.float8e4
AF = mybir.ActivationFunctionType
ALU = mybir.AluOpType
DR = mybir.MatmulPerfMode.DoubleRow

_CACHE = {}


def _build(inv_t: float, bias_den: float):
    nc = bacc.Bacc("TRN2", target_bir_lowering=False, debug=False,
                   num_devices=N_CORES)

    # xdr[p, g*KC*JG + k*JG + n] = xn^T[k*128+p, g*JG+n]
    xdr = nc.dram_tensor("xdr", [P, NG * KC * JG], F8, kind="ExternalInput")
    # wcc[p, :KC*RP] = proxy weights [k*RP+m]; [KC*RP:] = inst weights
    wcc = nc.dram_tensor("wcc", [P, 2 * KC * RP], F8, kind="ExternalInput")
    mk = nc.dram_tensor("mk", [RP, N_FULL], F8, kind="ExternalInput")
    # p-major output; host de-interleaves [p, it*8+c] -> [it*128+p, c]
    out = nc.dram_tensor("out", [P, 36], F32, kind="ExternalOutput")

    with tile.TileContext(nc) as tc:
        with (
            tc.tile_pool(name="xpool", bufs=1) as xpool,
            tc.tile_pool(name="lhs", bufs=1) as lhs,
            tc.tile_pool(name="maskp", bufs=1) as maskp,
            tc.tile_pool(name="zpool", bufs=6) as zpool,
            tc.tile_pool(name="zopool", bufs=2) as zopool,
            tc.tile_pool(name="small", bufs=1) as small,
            tc.tile_pool(name="ps", bufs=2, space="PSUM") as pspool,
        ):
            # ---- constants (no DMA deps; emitted first) ----
            zeros_w = small.tile([P, P], BF16, name="zeros_w")
            nc.vector.memset(zeros_w[:], 0.0)
            zeros_r = small.tile([P, JT], BF16, name="zeros_r")
            nc.vector.memset(zeros_r[:], 0.0)
            dummy = small.tile([P, 1], F32, name="dummy")
            nc.vector.memset(dummy[:], 0.0)
            # trigger the ~2.7us exp table-set load during the DMA preamble
            nc.scalar.activation(dummy[:], dummy[:], AF.Exp)

            # ---- loads: one ring (engines are shared), by first use ----
            wc = lhs.tile([P, 2 * KC * RP], F8, name="wc")
            xg = [xpool.tile([P, KC * JG], F8, name=f"xg{g}")
                  for g in range(NG)]
            mask_t = [maskp.tile([P, N_FULL], F8, name=f"mask{it}")
                      for it in range(IT)]
            W = KC * JG
            nc.sync.dma_start(wc[:], wcc.ap())
            nc.sync.dma_start(xg[0][:], xdr.ap()[:, 0:W])
            nc.sync.dma_start(mask_t[0][:], mk.ap()[0:P, :])
            nc.sync.dma_start(xg[1][:], xdr.ap()[:, W:2 * W])
            for it in range(1, IT):
                nc.sync.dma_start(mask_t[it][:],
                                  mk.ap()[it * P:(it + 1) * P, :])

            # 3D views for DoubleRow slicing: [P, k-chunk, cols]
            xg3 = [t[:].rearrange("p (k n) -> p k n", k=KC) for t in xg]
            wp3 = wc[:, 0:KC * RP].rearrange("p (k m) -> p k m", k=KC)
            wx3 = wc[:, KC * RP:].rearrange("p (k m) -> p k m", k=KC)

            # ---- main loop ----
            # acc columns: it*8 + mat*4 + [0/1]=den(g0,g1), [2/3]=num
            acc = small.tile([P, 36], F32, name="acc")

            first = True
            for it in range(IT):
                i0 = it * P
                for g in range(NG):
                    for mat in range(2):
                        w3 = wp3 if mat == 0 else wx3
                        ps = pspool.tile([P, JG], F32,
                                         name=f"ps_{it}_{mat}_{g}", tag="ps")
                        if first:
                            # HAM warm-up: zero matmuls keep the PE busy
                            # while input DMAs stream, so the clock is at
                            # 8/8 when the real stream begins.
                            for w in range(14):
                                nc.tensor.matmul(
                                    ps[:, 0:JT], zeros_w[:], zeros_r[:],
                                    start=(w == 0), stop=(w == 13),
                                )
                            first = False
                        for b in range(2):  # DoubleRow K-blocks (256 each)
                            ksl = slice(2 * b, 2 * b + 2)
                            for jl in range(JG // JT):
                                nc.tensor.matmul(
                                    ps[:, jl * JT:(jl + 1) * JT],
                                    w3[:, ksl, i0:i0 + P],
                                    xg3[g][:, ksl, jl * JT:(jl + 1) * JT],
                                    start=(b == 0), stop=(b == 1),
                                    perf_mode=DR,
                                )
                        cd = it * 8 + mat * 4 + g
                        anchor = (it == 0 and g == 0 and mat == 0)
                        halves = (2 if anchor else 1)
                        hw_ = JG // halves
                        for hh in range(halves):
                            z = zpool.tile([P, hw_], BF16,
                                           name=f"z_{it}_{mat}_{g}_{hh}",
                                           tag="z")
                            zo = zopool.tile([P, hw_], BF16,
                                             name=f"zo_{it}_{mat}_{g}_{hh}",
                                             tag="zo")
                            hd, hn = (cd, cd + 2) if hh == 0 else (32, 34)
                            nc.scalar.activation(
                                z[:], ps[:, hh * hw_:(hh + 1) * hw_], AF.Exp,
                                bias=bias_den, scale=inv_t,
                                accum_out=acc[:, hd:hd + 1],
                            )
                            m0 = g * JG + hh * hw_
                            nc.vector.scalar_tensor_tensor(
                                out=zo[:], in0=z[:], scalar=1.0,
                                in1=mask_t[it][:, m0:m0 + hw_],
                                op0=ALU.mult, op1=ALU.mult,
                                accum_out=acc[:, hn:hn + 1],
                            )
            nc.sync.dma_start(out.ap()[:], acc[:])

    nc.compile()
    return nc


def _chunked(aT):
    """[D, n] -> [128, KC * n] with free layout [k-chunk, col]."""
    return np.ascontiguousarray(
        aT.reshape(KC, P, -1).transpose(1, 0, 2).reshape(P, -1))


def make_in_maps(x, p, nmf, lab):
    eps = 1e-8
    xn = x / np.maximum(np.linalg.norm(x, axis=-1, keepdims=True), eps)
    pn = p / np.maximum(np.linalg.norm(p, axis=-1, keepdims=True), eps)
    xnT = xn.T.astype(ml_dtypes.float8_e4m3)
    pnT = pn.T.astype(ml_dtypes.float8_e4m3)
    # xdr free layout: [g, k, n]  (g = column group of JG)
    xdr = np.ascontiguousarray(
        xnT.reshape(KC, P, NG, JG).transpose(1, 2, 0, 3).reshape(P, -1))
    in_maps = []
    for c in range(N_CORES):
        rows = slice(c * RP, (c + 1) * RP)
        in_maps.append({
            "xdr": xdr,
            "wcc": np.concatenate(
                [_chunked(pnT[:, rows]), _chunked(xnT[:, rows])], axis=1),
            "mk": nmf[lab[rows]].astype(ml_dtypes.float8_e4m3),
        })
    return in_maps


def kernel(inst_embed, proxy, negative_mask, labels, temperature, margin):
    t = float(np.asarray(temperature))
    m = float(np.asarray(margin))
    inv_t = 1.0 / t
    bias_den = -m / t

    key = (t, m)
    if key not in _CACHE:
        _CACHE[key] = _build(inv_t, bias_den)
    nc = _CACHE[key]

    x = np.asarray(inst_embed, dtype=np.float32)
    p = np.asarray(proxy, dtype=np.float32)
    nmf = np.asarray(negative_mask, dtype=np.float32)
    lab = np.asarray(labels).astype(np.int64)

    in_maps = make_in_maps(x, p, nmf, lab)

    res = bass_utils.run_bass_kernel_spmd(nc, in_maps,
                                          core_ids=list(range(N_CORES)))
    # out is p-major [128, it*8+c] (+ anchor-split extras in cols 32/34)
    full = np.concatenate(
        [np.asarray(res.results[c]["out"]) for c in range(N_CORES)],
        axis=0).astype(np.float64)  # [8*128, 36]
    outs = np.zeros((N_FULL, 8))
    extra = np.zeros((N_FULL, 2))
    for c in range(N_CORES):
        o = full[c * P:(c + 1) * P]
        for it in range(IT):
            rows = slice(c * RP + it * P, c * RP + (it + 1) * P)
            outs[rows] = o[:, it * 8:(it + 1) * 8]
            if it == 0:
                extra[rows] = o[:, [32, 34]]
    den_p = outs[:, 0] + outs[:, 1] + extra[:, 0]
    num_p = outs[:, 2] + outs[:, 3] + extra[:, 1]
    den_i = outs[:, 4] + outs[:, 5]
    num_i = outs[:, 6] + outs[:, 7]
    loss = (-2.0 * np.log(t)
            + (np.log(den_p) - np.log(num_p)).mean()
            + (np.log(den_i) - np.log(num_i)).mean())
    return np.float32(loss)


# revision 36
# speedup vs baseline: 1.0596x; 1.0596x over previous
"""Conditional contrastive loss on 8 TRN2 NeuronCores (Bass/Tile).

Strategy (data-parallel over rows, per sharding hint):
  - Each core owns 512 rows (of 4096) of inst_embed ("x") and proxy ("p").
  - The host row-normalizes x and p in fp32 and ships fp8(e4m3) operands
    in the exact on-chip layout: the full normalized xn^T (matmul rhs,
    k-chunked, one tile per 2048-column group), the core's own xn/pn
    columns (matmul lhsT, both matrices in one 4KB-row tensor), and the
    core's pre-gathered positive-selection mask rows negative_mask[labels]
    (fp8; 0/1 exact). This removes the entire on-device normalization
    pipeline, and every DMA moves fully-contiguous 4KB rows (the DMA
    fabric moves fixed ~158ns 4KB packet-slots across 16 engines).
  - Similarity rows sim[i, j] for the core's i-block: fp8 DoubleRow
    matmuls (2 contraction rows per PE cell -> K=256 per instruction)
    accumulated in PSUM, 2048 columns per PSUM group, double-buffered.
    A zero-matmul warmup stream keeps the PE HAM-warm through the DMA
    preamble (the HAM reaches 8/8 mid-warmup and must not re-throttle
    before the first data lands).
  - exp((sim-margin)/T) on the scalar engine straight out of PSUM with
    accum_out = per-group row sums -> denominator; z to SBUF in bf16.
  - numerator = scalar_tensor_tensor(z * mask) on DVE with accum_out.
    The DVE 1x fused op is the steady-state gate (~2.21us per 2048-col
    group); TT/TS alternatives measure slower because the fp8 and accum
    variants have no 2x uops.
  - Device emits raw per-group (den, num) row sums (p-major [128, 32]
    f32, one DMA); the host does the final group-sum/log/mean.
"""
import numpy as np
import ml_dtypes

import concourse.bacc as bacc
import concourse.tile as tile
from concourse import mybir, bass_utils

N_FULL = 4096
D = 512
N_CORES = 8
RP = N_FULL // N_CORES  # rows per core = 512
P = 128                 # SBUF partitions
KC = D // P             # 128-row contraction chunks = 4
JT = 512                # columns per PSUM bank
JG = 2048               # columns per PSUM group (4 banks)
NG = N_FULL // JG       # groups per (i-tile, matrix) = 2
IT = RP // P            # i-tiles per core = 4

F32 = mybir.dt.float32
BF16 = mybir.dt.bfloat16
F8 = mybir.dt.float8e4
AF = mybir.ActivationFunctionType
ALU = mybir.AluOpType
DR = mybir.MatmulPerfMode.DoubleRow

_CACHE = {}


def _build(inv_t: float, bias_den: float):
    nc = bacc.Bacc("TRN2", target_bir_lowering=False, debug=False,
                   num_devices=N_CORES)

    # xdr[p, g*KC*JG + k*JG + n] = xn^T[k*128+p, g*JG+n]
    xdr = nc.dram_tensor("xdr", [P, NG * KC * JG], F8, kind="ExternalInput")
    # wcc[p, :KC*RP] = proxy weights [k*RP+m]; [KC*RP:] = inst weights
    wcc = nc.dram_tensor("wcc", [P, 2 * KC * RP], F8, kind="ExternalInput")
    mk = nc.dram_tensor("mk", [RP, N_FULL], F8, kind="ExternalInput")
    # p-major output; host de-interleaves [p, it*8+c] -> [it*128+p, c]
    out = nc.dram_tensor("out", [P, 4 * 8], F32, kind="ExternalOutput")

    with tile.TileContext(nc) as tc:
        with (
            tc.tile_pool(name="xpool", bufs=1) as xpool,
            tc.tile_pool(name="lhs", bufs=1) as lhs,
            tc.tile_pool(name="maskp", bufs=1) as maskp,
            tc.tile_pool(name="zpool", bufs=6) as zpool,
            tc.tile_pool(name="zopool", bufs=2) as zopool,
            tc.tile_pool(name="small", bufs=1) as small,
            tc.tile_pool(name="ps", bufs=2, space="PSUM") as pspool,
        ):
            # ---- constants (no DMA deps; emitted first) ----
            zeros_w = small.tile([P, P], BF16, name="zeros_w")
            nc.vector.memset(zeros_w[:], 0.0)
            zeros_r = small.tile([P, JT], BF16, name="zeros_r")
            nc.vector.memset(zeros_r[:], 0.0)
            dummy = small.tile([P, 1], F32, name="dummy")
            nc.vector.memset(dummy[:], 0.0)
            # trigger the ~2.7us exp table-set load during the DMA preamble
            nc.scalar.activation(dummy[:], dummy[:], AF.Exp)

            # ---- loads: one ring (engines are shared), by first use ----
            wc = lhs.tile([P, 2 * KC * RP], F8, name="wc")
            xg = [xpool.tile([P, KC * JG], F8, name=f"xg{g}")
                  for g in range(NG)]
            mask_t = [maskp.tile([P, N_FULL], F8, name=f"mask{it}")
                      for it in range(IT)]
            W = KC * JG
            nc.sync.dma_start(wc[:], wcc.ap())
            nc.sync.dma_start(xg[0][:], xdr.ap()[:, 0:W])
            nc.sync.dma_start(mask_t[0][:], mk.ap()[0:P, :])
            nc.sync.dma_start(xg[1][:], xdr.ap()[:, W:2 * W])
            for it in range(1, IT):
                nc.sync.dma_start(mask_t[it][:],
                                  mk.ap()[it * P:(it + 1) * P, :])

            # 3D views for DoubleRow slicing: [P, k-chunk, cols]
            xg3 = [t[:].rearrange("p (k n) -> p k n", k=KC) for t in xg]
            wp3 = wc[:, 0:KC * RP].rearrange("p (k m) -> p k m", k=KC)
            wx3 = wc[:, KC * RP:].rearrange("p (k m) -> p k m", k=KC)

            # ---- main loop ----
            # acc columns: it*8 + mat*4 + [0/1]=den(g0,g1), [2/3]=num
            acc = small.tile([P, 4 * 8], F32, name="acc")

            first = True
            for it in range(IT):
                i0 = it * P
                for g in range(NG):
                    for mat in range(2):
                        w3 = wp3 if mat == 0 else wx3
                        ps = pspool.tile([P, JG], F32,
                                         name=f"ps_{it}_{mat}_{g}", tag="ps")
                        if first:
                            # HAM warm-up: zero matmuls keep the PE busy
                            # while input DMAs stream, so the clock is at
                            # 8/8 when the real stream begins.
                            for w in range(14):
                                nc.tensor.matmul(
                                    ps[:, 0:JT], zeros_w[:], zeros_r[:],
                                    start=(w == 0), stop=(w == 13),
                                )
                            first = False
                        for b in range(2):  # DoubleRow K-blocks (256 each)
                            ksl = slice(2 * b, 2 * b + 2)
                            for jl in range(JG // JT):
                                nc.tensor.matmul(
                                    ps[:, jl * JT:(jl + 1) * JT],
                                    w3[:, ksl, i0:i0 + P],
                                    xg3[g][:, ksl, jl * JT:(jl + 1) * JT],
                                    start=(b == 0), stop=(b == 1),
                                    perf_mode=DR,
                                )
                        z = zpool.tile([P, JG], BF16,
                                       name=f"z_{it}_{mat}_{g}", tag="z")
                        zo = zopool.tile([P, JG], BF16,
                                         name=f"zo_{it}_{mat}_{g}", tag="zo")
                        cd = it * 8 + mat * 4 + g
                        nc.scalar.activation(
                            z[:], ps[:], AF.Exp,
                            bias=bias_den, scale=inv_t,
                            accum_out=acc[:, cd:cd + 1],
                        )
                        nc.vector.scalar_tensor_tensor(
                            out=zo[:], in0=z[:], scalar=1.0,
                            in1=mask_t[it][:, g * JG:(g + 1) * JG],
                            op0=ALU.mult, op1=ALU.mult,
                            accum_out=acc[:, cd + 2:cd + 3],
                        )
            nc.sync.dma_start(out.ap()[:], acc[:])

    nc.compile()
    return nc


def _chunked(aT):
    """[D, n] -> [128, KC * n] with free layout [k-chunk, col]."""
    return np.ascontiguousarray(
        aT.reshape(KC, P, -1).transpose(1, 0, 2).reshape(P, -1))


def make_in_maps(x, p, nmf, lab):
    eps = 1e-8
    xn = x / np.maximum(np.linalg.norm(x, axis=-1, keepdims=True), eps)
    pn = p / np.maximum(np.linalg.norm(p, axis=-1, keepdims=True), eps)
    xnT = xn.T.astype(ml_dtypes.float8_e4m3)
    pnT = pn.T.astype(ml_dtypes.float8_e4m3)
    # xdr free layout: [g, k, n]  (g = column group of JG)
    xdr = np.ascontiguousarray(
        xnT.reshape(KC, P, NG, JG).transpose(1, 2, 0, 3).reshape(P, -1))
    in_maps = []
    for c in range(N_CORES):
        rows = slice(c * RP, (c + 1) * RP)
        in_maps.append({
            "xdr": xdr,
            "wcc": np.concatenate(
                [_chunked(pnT[:, rows]), _chunked(xnT[:, rows])], axis=1),
            "mk": nmf[lab[rows]].astype(ml_dtypes.float8_e4m3),
        })
    return in_maps


def kernel(inst_embed, proxy, negative_mask, labels, temperature, margin):
    t = float(np.asarray(temperature))
    m = float(np.asarray(margin))
    inv_t = 1.0 / t
    bias_den = -m / t

    key = (t, m)
    if key not in _CACHE:
        _CACHE[key] = _build(inv_t, bias_den)
    nc = _CACHE[key]

    x = np.asarray(inst_embed, dtype=np.float32)
    p = np.asarray(proxy, dtype=np.float32)
    nmf = np.asarray(negative_mask, dtype=np.float32)
    lab = np.asarray(labels).astype(np.int64)

    in_maps = make_in_maps(x, p, nmf, lab)

    res = bass_utils.run_bass_kernel_spmd(nc, in_maps,
                                          core_ids=list(range(N_CORES)))
    # out is p-major [128, it*8+c]; de-interleave to [rows, 8]
    outs = np.concatenate(
        [np.asarray(res.results[c]["out"]).reshape(P, IT, 8)
         .transpose(1, 0, 2).reshape(RP, 8) for c in range(N_CORES)],
        axis=0).astype(np.float64)
    den_p = outs[:, 0] + outs[:, 1]
    num_p = outs[:, 2] + outs[:, 3]
    den_i = outs[:, 4] + outs[:, 5]
    num_i = outs[:, 6] + outs[:, 7]
    loss = (-2.0 * np.log(t)
            + (np.log(den_p) - np.log(num_p)).mean()
            + (np.log(den_i) - np.log(num_i)).mean())
    return np.float32(loss)
